# revision 41
# baseline (speedup 1.0000x reference)
"""DualReprogrammingLayer Trainium2 kernel.

Sharding: 2 row-groups (B*L split in halves) x 4 head-groups (4 heads each).
Each core computes, for its 2048 rows and 4 heads (per block in {trend, detail}):
  KT = (Wk.T @ protoT)           (heads-slice, S)
  V  = (protoT.T @ Wv) + bv      (S, heads-slice)
  qT = (Wq.T @ xT) + bq          (heads-slice, rows)
  scoresT = KT_h @ qT_h          (S, rows) per head   [2-head row-packed]
  P  = exp(scoresT / 8)
  A_ext = [V_h | ones].T @ P     (64+64, rows)        [rows 64:128 = denom]
  A_n = A / denom                                      [no gate on device]
  out_b_partial = A_b.T-chunks @ Wo_b  (2048, 4096) bf16, b in {t, d}
Gate MLP is row-sharded 8 ways instead of computed redundantly per core:
core c computes gate = sigmoid(relu(gx@W1)@W2) for global rows
[512c, 512c+512) only (gx is a per-core input slice).
Host: per row-group, Pt = sum_hg out_t, Pd = sum_hg out_d;
out = g*Pt + (1-g)*Pd (+ g*bo_t + (1-g)*bo_d), g assembled from 8 slices.
"""
import sys
sys.path.insert(0, '/opt/trn_rl_repo')
from contextlib import ExitStack

import numpy as np
import ml_dtypes

import concourse.bass as bass
import concourse.tile as tile
from concourse import bacc, mybir

F32 = mybir.dt.float32
BF16 = mybir.dt.bfloat16
AF = mybir.ActivationFunctionType
bf16 = ml_dtypes.bfloat16

B, L, D, S, DLLM, H, E = 4, 1024, 1024, 1000, 4096, 16, 64
RG, HG = 2, 4                 # row-groups x head-groups = 8 cores
R = (B * L) // RG             # 2048 rows per core
NH = H // HG                  # 4 heads per core
HEC = NH * E                  # 256
GR = (B * L) // 8             # 512 gate rows per core
SCH, NSC = 125, 8             # S = 8 chunks of 125
RC, NRC = 512, 4              # rows = 4 chunks of 512
KD = D // 128                 # 8 k-chunks for d_model
KL = DLLM // 128              # 32 k-chunks for d_llm

_CACHE = {}
LAST_RESULTS = None           # set by kernel(): BassKernelResults


def _build():
    nc = bacc.Bacc("TRN2", target_bir_lowering=False, debug=False)

    def din(name, shape, dt):
        return nc.dram_tensor(name, list(shape), dt, kind="ExternalInput")

    xT = {b: din(f"xT_{b}", (D, R), BF16) for b in "td"}
    pT = {b: din(f"pT_{b}", (DLLM, S), BF16) for b in "td"}
    wq = {b: din(f"wq_{b}", (D, HEC), BF16) for b in "td"}
    wk = {b: din(f"wk_{b}", (DLLM, HEC), BF16) for b in "td"}
    wv = {b: din(f"wv_{b}", (DLLM, HEC), BF16) for b in "td"}
    wo = din("wo", (2 * HEC, DLLM), BF16)            # [t rows | d rows]
    w1 = din("w1", (2 * D, D), BF16)
    w2 = din("w2", (D, 1), BF16)
    gx = din("gxT", (2 * D, GR), BF16)               # cat(x_t,x_d)[gate rows].T
    bq2 = din("bq2", (128, 4), F32)                  # cols: t-mc0, t-mc1, d-mc0, d-mc1
    bk2 = din("bk2", (128, 4), F32)
    bvv = din("bv", (1, 2 * HEC), BF16)              # [t 256 | d 256]
    gb1 = din("gb1", (128, KD), F32)
    gb2 = din("gb2", (1, 1), F32)
    out_dr = {b: nc.dram_tensor(f"out_{b}", [R, DLLM], BF16,
                                kind="ExternalOutput") for b in "td"}
    gate_dr = nc.dram_tensor("gate", [1, GR], F32, kind="ExternalOutput")

    with tile.TileContext(nc) as tc, ExitStack() as ctx:
        # ---- persistent pools (live across phases) ----
        pers = ctx.enter_context(tc.tile_pool(name="pers", bufs=1))
        kt_sb = {}    # block -> tile (128, 2, S) bf16 : HE chunk mc at [:, mc, :]
        vx_sb = {}    # block -> tile (125, NSC, NH, 65) bf16 : [V_h | ones]
        qt_sb = {}    # block -> tile (128, 2, R) bf16
        for b in "td":
            kt_sb[b] = pers.tile([128, 2, S], BF16, tag=f"kt_{b}", name=f"kt_{b}")
            vx_sb[b] = pers.tile([SCH, NSC, NH, 65], BF16, tag=f"vx_{b}", name=f"vx_{b}")
            qt_sb[b] = pers.tile([128, 2, R], BF16, tag=f"qt_{b}", name=f"qt_{b}")
        ones125 = pers.tile([1, SCH], BF16, tag="ones125")
        nc.vector.memset(ones125[:], 1.0)
        bq_sb = pers.tile([128, 4], F32, tag="bq")
        nc.gpsimd.dma_start(bq_sb[:], bq2.ap())
        bk_sb = pers.tile([128, 4], F32, tag="bk")
        nc.gpsimd.dma_start(bk_sb[:], bk2.ap())
        bv_sb = pers.tile([1, 2 * HEC], BF16, tag="bv")
        nc.gpsimd.dma_start(bv_sb[:], bvv.ap())
        gb1_sb = pers.tile([128, KD], F32, tag="gb1")
        nc.gpsimd.dma_start(gb1_sb[:], gb1.ap())
        gb2_sb = pers.tile([1, 1], F32, tag="gb2")
        nc.gpsimd.dma_start(gb2_sb[:], gb2.ap())

        # ---- prefetch pools. p_pre1 (wq) is loaded during phase A; the
        # large phase-C weights (wo/w1/gx/w2) are allocated and loaded at
        # phase B start to keep phase-A SBUF small. Both live through C.
        pre1ctx = ExitStack()
        p_pre1 = pre1ctx.enter_context(tc.tile_pool(name="p_pre1", bufs=1))
        wq_t = {}
        for b in "td":
            wq_t[b] = p_pre1.tile([128, KD, HEC], BF16, tag=f"wq_{b}", name=f"wq_{b}")
        # rc0's x tiles, prefetched during phase A so phase B starts hot;
        # the pool is reused for the deferred last rows-chunk load in C
        p_x0 = pre1ctx.enter_context(tc.tile_pool(name="p_x0", bufs=1))
        xt0 = {b: p_x0.tile([128, KD, RC], BF16, tag=f"x0_{b}", name=f"x0_{b}")
               for b in "td"}

        # ---- phase A: K/V projections (proto and weights streamed per k-chunk) ----
        with ExitStack() as actx:
            p_pt = actx.enter_context(tc.tile_pool(name="p_pt", bufs=8))
            p_wc = actx.enter_context(tc.tile_pool(name="p_wc", bufs=1))
            psA = actx.enter_context(tc.tile_pool(name="psA", bufs=1, space="PSUM"))
            # PE warm-up: ~8us of dummy matmuls on memset tiles so the HAM
            # clock-gate reaches 8/8 before the first real matmul, overlapping
            # the initial DMA ramp. Results are discarded.
            wu_l = p_wc.tile([64, 128], BF16, tag="wu_l")
            nc.vector.memset(wu_l[:], 0.0)
            wu_r = p_wc.tile([64, 512], BF16, tag="wu_r")
            nc.vector.memset(wu_r[:], 0.0)

            # K/V weights: both blocks' tiles live through phase A; loads are
            # issued in 4-chunk (1 MB) groups with >=8-chunk lookahead so no
            # burst starves the pT stream
            wkv = {b: (p_wc.tile([128, KL, HEC], BF16, tag=f"wk_{b}", name=f"wk_{b}"),
                       p_wc.tile([128, KL, HEC], BF16, tag=f"wv_{b}", name=f"wv_{b}"))
                   for b in "td"}
            wkv_r = {b: (wk[b].ap().rearrange("(c p) m -> p c m", p=128),
                         wv[b].ap().rearrange("(c p) m -> p c m", p=128))
                     for b in "td"}
            gq = [(b, g) for b in "td" for g in range(KL // 4)]
            gqi = [0]

            def issue_wkv_group(n=1):
                for _ in range(n):
                    if gqi[0] >= len(gq):
                        return
                    gb, g = gq[gqi[0]]
                    gqi[0] += 1
                    hq = slice(g * 4, (g + 1) * 4)
                    nc.scalar.dma_start(wkv[gb][0][:, hq, :], wkv_r[gb][0][:, hq, :])
                    nc.scalar.dma_start(wkv[gb][1][:, hq, :], wkv_r[gb][1][:, hq, :])

            issue_wkv_group(2)

            for b in "td":
                vps = [psA.tile([SCH, 2, HEC], F32, tag=f"vps{i}", name=f"vps{i}")
                       for i in range(4)]
                kps = [psA.tile([128, 512], F32, tag=f"kps{i}", name=f"kps{i}")
                       for i in range(4)]
                if b == "t":
                    # discarded warm-up group in kps[0]'s bank, closed before
                    # the real accumulation group opens
                    for wi in range(16):
                        nc.tensor.matmul(kps[0][:], wu_l[:], wu_r[:],
                                         start=(wi == 0), stop=(wi == 15))
                wk_t, wv_t = wkv[b]
                pt_r = pT[b].ap().rearrange("(c p) s -> c p s", c=KL)
                for kc in range(KL):
                    if kc % 4 == 0:
                        issue_wkv_group(1)
                    if b == "d" and kc == 28:
                        for bb in "td":
                            nc.gpsimd.dma_start(
                                xt0[bb][:],
                                xT[bb].ap().rearrange("(c p) n -> p c n",
                                                      p=128)[:, :, 0:RC])
                    pt_t = p_pt.tile([128, S], BF16, tag="pt")
                    nc.sync.dma_start(pt_t[:], pt_r[kc])
                    wkc = wk_t[:, kc, :]
                    wvc = wv_t[:, kc, :]
                    for si in range(NSC):
                        # one accumulation group per PSUM bank: only the first
                        # half issues start=True (bank-wide clear covers both)
                        nc.tensor.matmul(
                            vps[si // 2][:, si % 2, :],
                            pt_t[:, si * SCH:(si + 1) * SCH],
                            wvc,
                            start=(kc == 0 and si % 2 == 0), stop=False)
                    for mc in range(2):
                        for ncc in range(2):
                            nc.tensor.matmul(
                                kps[mc * 2 + ncc][:, 0:500],
                                wk_t[:, kc, mc * 128:(mc + 1) * 128],
                                pt_t[:, ncc * 500:(ncc + 1) * 500],
                                start=(kc == 0), stop=(kc == KL - 1))
                boff = 0 if b == "t" else HEC
                for si in range(NSC):
                    nc.tensor.matmul(
                        vps[si // 2][:, si % 2, :],
                        ones125[:],
                        bv_sb[:, boff:boff + HEC],
                        start=False, stop=(si % 2 == 1))
                for si in range(NSC):
                    # copy V psum (125, 256) -> [:, si, :, 0:64] viewed as (125, 4, 64)
                    nc.vector.tensor_copy(
                        vx_sb[b][:, si, :, 0:64],
                        vps[si // 2][:, si % 2, :].rearrange("p (h e) -> p h e", h=NH))
                nc.vector.memset(vx_sb[b][:, :, :, 64:65], 1.0)
                for mc in range(2):
                    for ncc in range(2):
                        nc.scalar.activation(
                            kt_sb[b][:, mc, ncc * 500:(ncc + 1) * 500],
                            kps[mc * 2 + ncc][:, 0:500],
                            AF.Identity,
                            bias=bk_sb[:, (0 if b == "t" else 2) + mc:
                                       (0 if b == "t" else 2) + mc + 1])
                if b == "t":
                    # Q weights (needed at phase B start)
                    for bb in "td":
                        nc.scalar.dma_start(
                            wq_t[bb][:],
                            wq[bb].ap().rearrange("(c p) m -> p c m", p=128))

        # ---- phase B: Q projections (per rows-chunk; last chunk deferred) ----
        # Large phase-C weights: allocated after phase A's pools released,
        # streamed on the scalar queue from here; first consumers run at
        # rows-chunk 1 of phase C.
        p_pre2 = pre1ctx.enter_context(tc.tile_pool(name="p_pre2", bufs=1))
        wo_t = p_pre2.tile([128, 4, DLLM], BF16, tag="wo")
        w1_t = p_pre2.tile([128, 2 * KD, D], BF16, tag="w1")
        w2_t = p_pre2.tile([128, KD, 1], BF16, tag="w2")
        gx_t = p_pre2.tile([128, 2 * KD, GR], BF16, tag="gx")
        # engine-driven DMAs occupy their engine: sync is idle from the end
        # of the pT stream until the first out-DMA well into phase C; scalar
        # (Q-proj activations) and gpsimd (xt loads) are not
        nc.sync.dma_start(w1_t[:], w1.ap().rearrange("(c p) m -> p c m", p=128))
        nc.sync.dma_start(gx_t[:], gx.ap().rearrange("(c p) m -> p c m", p=128))
        nc.sync.dma_start(w2_t[:], w2.ap().rearrange("(c p) m -> p c m", p=128))
        nc.sync.dma_start(wo_t[:], wo.ap().rearrange("(c p) n -> p c n", p=128))
        with ExitStack() as bctx:
            p_x = bctx.enter_context(tc.tile_pool(name="p_x", bufs=2))
            psB = bctx.enter_context(tc.tile_pool(name="psB", bufs=2, space="PSUM"))
            for r in range(NRC - 1):
                rsl = slice(r * RC, (r + 1) * RC)
                if r == 0:
                    xt = xt0   # prefetched during phase A
                else:
                    xt = {}
                    for b in "td":
                        xt[b] = p_x.tile([128, KD, RC], BF16, tag=f"x_{b}",
                                         name=f"x_{b}")
                        nc.gpsimd.dma_start(
                            xt[b][:],
                            xT[b].ap().rearrange("(c p) n -> p c n",
                                                 p=128)[:, :, rsl])
                for b in "td":
                    for mc in range(2):
                        qps = psB.tile([128, RC], F32, tag="qps")
                        for kc in range(KD):
                            nc.tensor.matmul(
                                qps[:],
                                wq_t[b][:, kc, mc * 128:(mc + 1) * 128],
                                xt[b][:, kc, :],
                                start=(kc == 0), stop=(kc == KD - 1))
                        nc.scalar.activation(
                            qt_sb[b][:, mc, rsl], qps[:], AF.Identity,
                            bias=bq_sb[:, (0 if b == "t" else 2) + mc:
                                       (0 if b == "t" else 2) + mc + 1])

        # ---- phase C: attention + output projection ----
        # Software pipeline: QK+exp of unit u overlaps PV of unit u-1 at
        # s-chunk granularity; output-projection groups of the previous
        # rows-chunk (plus deferred Q-proj and the gate MLP) interleave
        # between units to keep the PE stream dense.
        with ExitStack() as cctx:
            p_p = cctx.enter_context(tc.tile_pool(name="p_p", bufs=5))
            p_a = cctx.enter_context(tc.tile_pool(name="p_a", bufs=2))
            p_s = cctx.enter_context(tc.tile_pool(name="p_s", bufs=2))
            p_o = cctx.enter_context(tc.tile_pool(name="p_o", bufs=2))
            p_h = cctx.enter_context(tc.tile_pool(name="p_h", bufs=1))
            # psO first so the attention pools (psS/psPV) can be released
            # before the tail, freeing their banks for a deeper tail pool
            psO = cctx.enter_context(tc.tile_pool(name="psO", bufs=2, space="PSUM"))
            attctx = ExitStack()
            psS = attctx.enter_context(tc.tile_pool(name="psS", bufs=1, space="PSUM"))
            psPV = attctx.enter_context(tc.tile_pool(name="psPV", bufs=1, space="PSUM"))

            def emit_qk_exp(b, mc, si, rsl):
                sps2 = psS.tile([SCH, 2, RC], F32, tag="sps", name="sps")
                p2 = p_p.tile([SCH, 2, RC], BF16, tag=f"p{si % 2}",
                              name=f"p{si % 2}")
                # exp split per packed head: each half starts right after its
                # QK matmul and releases the sps bank region sooner
                for hh in range(2):
                    po = hh * 64
                    nc.tensor.matmul(
                        sps2[:, hh, :],
                        kt_sb[b][po:po + 64, mc, si * SCH:(si + 1) * SCH],
                        qt_sb[b][po:po + 64, mc, rsl],
                        start=True, stop=True,
                        tile_position=(po, 0))
                    nc.scalar.activation(p2[:, hh, :], sps2[:, hh, :],
                                         AF.Exp, scale=0.125)
                return p2

            def emit_pv(aps, b, mc, si, p2):
                for hh in range(2):
                    h = mc * 2 + hh
                    nc.tensor.matmul(
                        aps[hh][:], vx_sb[b][:, si, h, :], p2[:, hh, :],
                        start=(si == 0), stop=(si == NSC - 1))

            def emit_norm(aps, b, mc, a2, rsl):
                for hh in range(2):
                    den1 = p_s.tile([1, RC], F32, tag="den1")
                    nc.vector.tensor_copy(den1[:], aps[hh][64:65, :])
                    rec1 = p_s.tile([1, RC], F32, tag="rec1")
                    nc.vector.reciprocal_approx_fast(rec1[:], den1[:])
                    sct64 = p_s.tile([64, RC], F32, tag="sct64")
                    nc.gpsimd.partition_broadcast(sct64[:], rec1[:])
                    nc.vector.tensor_mul(
                        a2[b][mc][hh * 64:hh * 64 + 64, :],
                        aps[hh][0:64, :], sct64[:])

            # deferred phase-B work for the last rows-chunk, plus the gate
            # MLP for this core's 512-row slice — PE filler with no
            # PSUM->SBUF copy load, spread across the rows-chunks
            ht = p_h.tile([128, KD, GR], BF16, tag="ht")
            bstate = {}

            def emit_b3_load(rB):
                rslB = slice(rB * RC, (rB + 1) * RC)
                xtB = {}
                for b in "td":
                    # reuses the rc0-prefetch pool (its phase-B reads are done)
                    xtB[b] = p_x0.tile([128, KD, RC], BF16, tag=f"x0_{b}",
                                       name=f"xB_{b}")
                    nc.gpsimd.dma_start(
                        xtB[b][:],
                        xT[b].ap().rearrange("(c p) n -> p c n", p=128)[:, :, rslB])
                bstate[rB] = xtB

            def emit_b3_q(rB, bb, mcq):
                rslB = slice(rB * RC, (rB + 1) * RC)
                xtB = bstate[rB]
                qps = psO.tile([128, RC], F32, tag="ops", name="qpsB")
                for kc in range(KD):
                    nc.tensor.matmul(
                        qps[:],
                        wq_t[bb][:, kc, mcq * 128:(mcq + 1) * 128],
                        xtB[bb][:, kc, :],
                        start=(kc == 0), stop=(kc == KD - 1))
                nc.scalar.activation(
                    qt_sb[bb][:, mcq, rslB], qps[:], AF.Identity,
                    bias=bq_sb[:, (0 if bb == "t" else 2) + mcq:
                               (0 if bb == "t" else 2) + mcq + 1])

            def emit_gate_hidden(mc):
                hps = psO.tile([128, GR], F32, tag="ops", name="hpsG")
                for kc in range(2 * KD):
                    nc.tensor.matmul(
                        hps[:],
                        w1_t[:, kc, mc * 128:(mc + 1) * 128],
                        gx_t[:, kc, :],
                        start=(kc == 0), stop=(kc == 2 * KD - 1))
                nc.scalar.activation(
                    ht[:, mc, :], hps[:], AF.Relu, bias=gb1_sb[:, mc:mc + 1])

            def emit_gate_logit():
                lpst = psO.tile([128, GR], F32, tag="ops", name="lpsG")
                for mc in range(KD):
                    nc.tensor.matmul(
                        lpst[0:1, :], w2_t[:, mc, :], ht[:, mc, :],
                        start=(mc == 0), stop=(mc == KD - 1))
                gate_sb = pers.tile([1, GR], F32, tag="gate")
                nc.scalar.activation(gate_sb[:], lpst[0:1, :], AF.Sigmoid,
                                     bias=gb2_sb[:])
                nc.sync.dma_start(gate_dr.ap(), gate_sb[:])

            gtasks = [lambda: emit_b3_load(NRC - 1)]
            for bb in "td":
                for mcq in range(2):
                    gtasks.append(
                        lambda bb=bb, mcq=mcq: emit_b3_q(NRC - 1, bb, mcq))
            for mc in range(KD):
                gtasks.append(lambda mc=mc: emit_gate_hidden(mc))
            gtasks.append(emit_gate_logit)
            # rows-chunk r consumes gtasks[gshare[r]:gshare[r+1]] as extra
            # filler: rc0 the deferred Q-proj (before rc3 needs it), then the
            # gate spread over rc1-rc3
            gshare = [0, 9, 11, 13, 14]

            # copy-engine rotation for output-projection PSUM->SBUF moves:
            # GpSimd cannot read PSUM; Act also carries the exp stream
            cp_engines = [nc.vector, nc.vector, nc.scalar]

            def make_outproj_tasks(a2p, r_prev, pool=None, rot=None, tag="ops"):
                """One task per PSUM group; osb assembled per (bb, rb, half)."""
                state = {}
                cnt = [0]
                pool_ = pool or psO
                rot_ = rot or cp_engines

                def group(bb, rb, ncc):
                    row0 = r_prev * RC + rb * 128
                    half, nh = divmod(ncc, 4)
                    if nh == 0:
                        state[(bb, rb, half)] = p_o.tile(
                            [128, DLLM // 2], BF16, tag=f"osb_{bb}", name="osb")
                    osb = state[(bb, rb, half)]
                    nsl = slice(ncc * 512, (ncc + 1) * 512)
                    ops = pool_.tile([128, 512], F32, tag=tag, name="ops")
                    ko = 0 if bb == "t" else 2
                    for mcc in range(2):
                        nc.tensor.matmul(
                            ops[:], a2p[bb][mcc][:, rb * 128:(rb + 1) * 128],
                            wo_t[:, ko + mcc, nsl],
                            start=(mcc == 0), stop=(mcc == 1))
                    eng = rot_[cnt[0] % len(rot_)]
                    cnt[0] += 1
                    if eng is nc.scalar:
                        eng.activation(osb[:, nh * 512:(nh + 1) * 512], ops[:],
                                       AF.Identity)
                    else:
                        eng.tensor_copy(osb[:, nh * 512:(nh + 1) * 512], ops[:])
                    if nh == 3:
                        nc.sync.dma_start(
                            out_dr[bb].ap()[row0:row0 + 128,
                                            half * 2048:(half + 1) * 2048],
                            osb[:])

                return [(lambda bb=bb, rb=rb, ncc=ncc: group(bb, rb, ncc))
                        for rb in range(4) for ncc in range(8) for bb in "td"]

            pending = None   # (a2 dict, r) awaiting output projection
            for r in range(NRC):
                rsl = slice(r * RC, (r + 1) * RC)
                a2 = {b: [p_a.tile([128, RC], BF16, tag=f"a2_{b}{mc}",
                                   name=f"a2_{b}{mc}")
                          for mc in range(2)] for b in "td"}
                units = [(b, mc) for b in "td" for mc in range(2)]
                # filler: out-proj groups of r-1 plus this rows-chunk's share
                # of the deferred Q/gate queue
                tasks = list(gtasks[gshare[r]:gshare[r + 1]])
                if pending is not None:
                    tasks = make_outproj_tasks(pending[0], pending[1]) + tasks
                ti = 0
                nslots = len(units) * NSC
                prev = None   # (aps, b, mc, p2dict)
                slot = 0
                for u, (b, mc) in enumerate(units):
                    aps = [psPV.tile([65, RC], F32, tag=f"aps{mc}{hh}",
                                     name=f"aps{mc}{hh}") for hh in range(2)]
                    p2buf = {}
                    for si in range(NSC):
                        if prev is not None:
                            paps, pb, pmc, pp2 = prev
                            emit_pv(paps, pb, pmc, si, pp2[si])
                        p2buf[si] = emit_qk_exp(b, mc, si, rsl)
                        # paced filler: one task per due slot
                        slot += 1
                        due = (slot * len(tasks)) // nslots
                        while ti < due:
                            tasks[ti]()
                            ti += 1
                    if prev is not None:
                        emit_norm(prev[0], prev[1], prev[2], a2, rsl)
                    prev = (aps, b, mc, p2buf)
                # drain last unit of this rows-chunk
                paps, pb, pmc, pp2 = prev
                for si in range(NSC):
                    emit_pv(paps, pb, pmc, si, pp2[si])
                emit_norm(paps, pb, pmc, a2, rsl)
                while ti < len(tasks):
                    tasks[ti]()
                    ti += 1
                pending = (a2, r)
            # tail: attention pools released; a 6-bank pool and an even
            # DVE/Act rotation keep the final out-projection copy-unbound
            attctx.close()
            psT = cctx.enter_context(tc.tile_pool(name="psT", bufs=6,
                                                  space="PSUM"))
            for task in make_outproj_tasks(pending[0], pending[1], pool=psT,
                                           rot=[nc.vector, nc.scalar],
                                           tag="ops2"):
                task()
        pre1ctx.close()

    nc.compile()
    return nc


def _prep_inputs(inputs):
    """Host-side shard + transpose. Returns in_maps for 8 cores."""
    f32 = np.float32
    t = {k: np.asarray(v) for k, v in inputs.items()}
    x_full = {"t": t["trend_emb"].reshape(B * L, D).astype(f32),
              "d": t["detail_emb"].reshape(B * L, D).astype(f32)}
    pT_full = {"t": np.ascontiguousarray(t["trend_proto"].astype(f32).T).astype(bf16),
               "d": np.ascontiguousarray(t["detail_proto"].astype(f32).T).astype(bf16)}
    cat = np.concatenate([x_full["t"], x_full["d"]], axis=1)  # (B*L, 2D)
    w1_bf = t["g_W1"].astype(bf16)
    w2_bf = t["g_W2"].astype(bf16)
    gb1_f = np.ascontiguousarray(t["g_b1"].astype(f32).reshape(KD, 128).T)
    gb2_f = t["g_b2"].astype(f32).reshape(1, 1)
    W = {("q", "t"): t["t_Wq"], ("q", "d"): t["d_Wq"],
         ("k", "t"): t["t_Wk"], ("k", "d"): t["d_Wk"],
         ("v", "t"): t["t_Wv"], ("v", "d"): t["d_Wv"],
         ("o", "t"): t["t_Wo"], ("o", "d"): t["d_Wo"]}
    bias = {("q", "t"): t["t_bq"], ("q", "d"): t["d_bq"],
            ("k", "t"): t["t_bk"], ("k", "d"): t["d_bk"],
            ("v", "t"): t["t_bv"], ("v", "d"): t["d_bv"],
            ("o", "t"): t["t_bo"], ("o", "d"): t["d_bo"]}

    in_maps = []
    for core in range(8):
        rg, hg = divmod(core, HG)
        rows = slice(rg * R, (rg + 1) * R)
        hsl = slice(hg * HEC, (hg + 1) * HEC)
        gsl = slice(core * GR, (core + 1) * GR)
        m = {}
        for b in "td":
            m[f"xT_{b}"] = np.ascontiguousarray(x_full[b][rows].T).astype(bf16)
            m[f"pT_{b}"] = pT_full[b]
            m[f"wq_{b}"] = np.ascontiguousarray(W[("q", b)][:, hsl]).astype(bf16)
            m[f"wk_{b}"] = np.ascontiguousarray(W[("k", b)][:, hsl]).astype(bf16)
            m[f"wv_{b}"] = np.ascontiguousarray(W[("v", b)][:, hsl]).astype(bf16)
        m["wo"] = np.vstack([W[("o", "t")][hsl, :], W[("o", "d")][hsl, :]]).astype(bf16)
        m["w1"] = w1_bf
        m["w2"] = w2_bf
        m["gxT"] = np.ascontiguousarray(cat[gsl].T).astype(bf16)
        m["bq2"] = np.stack([bias[("q", "t")][hsl][0:128], bias[("q", "t")][hsl][128:256],
                             bias[("q", "d")][hsl][0:128], bias[("q", "d")][hsl][128:256]],
                            axis=1).astype(f32)
        m["bk2"] = np.stack([bias[("k", "t")][hsl][0:128], bias[("k", "t")][hsl][128:256],
                             bias[("k", "d")][hsl][0:128], bias[("k", "d")][hsl][128:256]],
                            axis=1).astype(f32)
        m["bv"] = np.concatenate([bias[("v", "t")][hsl],
                                  bias[("v", "d")][hsl]])[None, :].astype(bf16)
        m["gb1"] = gb1_f
        m["gb2"] = gb2_f
        in_maps.append(m)
    bo = (np.asarray(bias[("o", "t")], f32), np.asarray(bias[("o", "d")], f32))
    return in_maps, bo


def kernel(**inputs):
    global LAST_RESULTS
    import os
    from concourse.bass_utils import run_bass_kernel_spmd

    in_maps, (bo_t, bo_d) = _prep_inputs(inputs)
    if "nc" not in _CACHE:
        _CACHE["nc"] = _build()
    nc = _CACHE["nc"]

    trace = bool(os.environ.get("KERNEL_TRACE"))
    res = run_bass_kernel_spmd(
        nc, in_maps, list(range(8)),
        trace=trace, trace_cores=list(range(8)) if trace else None)
    LAST_RESULTS = res

    f32 = np.float32
    g = np.concatenate([np.asarray(res.results[c]["gate"]).reshape(GR)
                        for c in range(8)]).astype(f32)[:, None]   # (B*L, 1)
    out = np.empty((RG, R, DLLM), f32)
    for rg in range(RG):
        Pt = res.results[rg * HG]["out_t"].astype(f32)
        Pd = res.results[rg * HG]["out_d"].astype(f32)
        for hg in range(1, HG):
            Pt += res.results[rg * HG + hg]["out_t"].astype(f32)
            Pd += res.results[rg * HG + hg]["out_d"].astype(f32)
        grg = g[rg * R:(rg + 1) * R]
        out[rg] = grg * Pt + (1.0 - grg) * Pd
        if np.any(bo_t) or np.any(bo_d):
            out[rg] += grg * bo_t[None, :] + (1.0 - grg) * bo_d[None, :]
    return out.reshape(B, L, DLLM)


# revision 42
# speedup vs baseline: 1.2310x; 1.2310x over previous
"""DualReprogrammingLayer Trainium2 kernel.

Sharding: 2 row-groups (B*L split in halves) x 4 head-groups (4 heads each).
Each core computes, for its 2048 rows and 4 heads (per block in {trend, detail}):
  KT = (Wk.T @ protoT)           (heads-slice, S)
  V  = (protoT.T @ Wv) + bv      (S, heads-slice)
  qT = (Wq.T @ xT) + bq          (heads-slice, rows)
  scoresT = KT_h @ qT_h          (S, rows) per head   [2-head row-packed]
  P  = exp(scoresT / 8)
  A_ext = [V_h | ones].T @ P     (64+64, rows)        [rows 64:128 = denom]
  A_n = A / denom                                      [no gate on device]
  out_b_partial = A_b.T-chunks @ Wo_b  (2048, 4096) bf16, b in {t, d}
Gate MLP is row-sharded 8 ways instead of computed redundantly per core:
core c computes gate = sigmoid(relu(gx@W1)@W2) for global rows
[512c, 512c+512) only (gx is a per-core input slice).
Host: per row-group, Pt = sum_hg out_t, Pd = sum_hg out_d;
out = g*Pt + (1-g)*Pd (+ g*bo_t + (1-g)*bo_d), g assembled from 8 slices.
"""
import sys
sys.path.insert(0, '/opt/trn_rl_repo')
from contextlib import ExitStack

import numpy as np
import ml_dtypes

import concourse.bass as bass
import concourse.tile as tile
from concourse import bacc, mybir

F32 = mybir.dt.float32
BF16 = mybir.dt.bfloat16
AF = mybir.ActivationFunctionType
bf16 = ml_dtypes.bfloat16

B, L, D, S, DLLM, H, E = 4, 1024, 1024, 1000, 4096, 16, 64
RG, HG = 2, 4                 # row-groups x head-groups = 8 cores
R = (B * L) // RG             # 2048 rows per core
NH = H // HG                  # 4 heads per core
HEC = NH * E                  # 256
GR = (B * L) // 8             # 512 gate rows per core
SCH, NSC = 125, 8             # S = 8 chunks of 125
RC, NRC = 512, 4              # rows = 4 chunks of 512
KD = D // 128                 # 8 k-chunks for d_model
KL = DLLM // 128              # 32 k-chunks for d_llm

_CACHE = {}
LAST_RESULTS = None           # set by kernel(): BassKernelResults


def _build():
    nc = bacc.Bacc("TRN2", target_bir_lowering=False, debug=False)

    def din(name, shape, dt):
        return nc.dram_tensor(name, list(shape), dt, kind="ExternalInput")

    xT = {b: din(f"xT_{b}", (D, R), BF16) for b in "td"}
    pT = {b: din(f"pT_{b}", (DLLM, S), BF16) for b in "td"}
    wq = {b: din(f"wq_{b}", (D, HEC), BF16) for b in "td"}
    wk = {b: din(f"wk_{b}", (DLLM, HEC), BF16) for b in "td"}
    wv = {b: din(f"wv_{b}", (DLLM, HEC), BF16) for b in "td"}
    wo = din("wo", (2 * HEC, DLLM), BF16)            # [t rows | d rows]
    w1 = din("w1", (2 * D, D), BF16)
    w2 = din("w2", (D, 1), BF16)
    gx = din("gxT", (2 * D, GR), BF16)               # cat(x_t,x_d)[gate rows].T
    bq2 = din("bq2", (128, 4), F32)                  # cols: t-mc0, t-mc1, d-mc0, d-mc1
    bk2 = din("bk2", (128, 4), F32)
    bvv = din("bv", (1, 2 * HEC), BF16)              # [t 256 | d 256]
    gb1 = din("gb1", (128, KD), F32)
    gb2 = din("gb2", (1, 1), F32)
    out_dr = {b: nc.dram_tensor(f"out_{b}", [R, DLLM], BF16,
                                kind="ExternalOutput") for b in "td"}
    gate_dr = nc.dram_tensor("gate", [1, GR], F32, kind="ExternalOutput")

    with tile.TileContext(nc) as tc, ExitStack() as ctx:
        # ---- persistent pools (live across phases) ----
        pers = ctx.enter_context(tc.tile_pool(name="pers", bufs=1))
        kt_sb = {}    # block -> tile (128, 2, S) bf16 : HE chunk mc at [:, mc, :]
        vx_sb = {}    # block -> tile (125, NSC, NH, 65) bf16 : [V_h | ones]
        qt_sb = {}    # block -> tile (128, 2, R) bf16
        for b in "td":
            kt_sb[b] = pers.tile([128, 2, S], BF16, tag=f"kt_{b}", name=f"kt_{b}")
            vx_sb[b] = pers.tile([SCH, NSC, NH, 65], BF16, tag=f"vx_{b}", name=f"vx_{b}")
            qt_sb[b] = pers.tile([128, 2, R], BF16, tag=f"qt_{b}", name=f"qt_{b}")
        ones125 = pers.tile([1, SCH], BF16, tag="ones125")
        nc.vector.memset(ones125[:], 1.0)
        bq_sb = pers.tile([128, 4], F32, tag="bq")
        nc.gpsimd.dma_start(bq_sb[:], bq2.ap())
        bk_sb = pers.tile([128, 4], F32, tag="bk")
        nc.gpsimd.dma_start(bk_sb[:], bk2.ap())
        bv_sb = pers.tile([1, 2 * HEC], BF16, tag="bv")
        nc.gpsimd.dma_start(bv_sb[:], bvv.ap())
        gb1_sb = pers.tile([128, KD], F32, tag="gb1")
        nc.gpsimd.dma_start(gb1_sb[:], gb1.ap())
        gb2_sb = pers.tile([1, 1], F32, tag="gb2")
        nc.gpsimd.dma_start(gb2_sb[:], gb2.ap())

        # ---- prefetch pools. p_pre1 (wq) is loaded during phase A; the
        # large phase-C weights (wo/w1/gx/w2) are allocated and loaded at
        # phase B start to keep phase-A SBUF small. Both live through C.
        pre1ctx = ExitStack()
        p_pre1 = pre1ctx.enter_context(tc.tile_pool(name="p_pre1", bufs=1))
        wq_t = {}
        for b in "td":
            wq_t[b] = p_pre1.tile([128, KD, HEC], BF16, tag=f"wq_{b}", name=f"wq_{b}")
        # rc0's x tiles, prefetched during phase A so phase B starts hot;
        # the pool is reused for the deferred last rows-chunk load in C
        p_x0 = pre1ctx.enter_context(tc.tile_pool(name="p_x0", bufs=1))
        xt0 = {b: p_x0.tile([128, KD, RC], BF16, tag=f"x0_{b}", name=f"x0_{b}")
               for b in "td"}

        # ---- phase A: K/V projections (proto and weights streamed per k-chunk) ----
        with ExitStack() as actx:
            p_pt = actx.enter_context(tc.tile_pool(name="p_pt", bufs=8))
            p_wc = actx.enter_context(tc.tile_pool(name="p_wc", bufs=1))
            psA = actx.enter_context(tc.tile_pool(name="psA", bufs=1, space="PSUM"))
            # PE warm-up: ~8us of dummy matmuls on memset tiles so the HAM
            # clock-gate reaches 8/8 before the first real matmul, overlapping
            # the initial DMA ramp. Results are discarded.
            wu_l = p_wc.tile([64, 128], BF16, tag="wu_l")
            nc.vector.memset(wu_l[:], 0.0)
            wu_r = p_wc.tile([64, 512], BF16, tag="wu_r")
            nc.vector.memset(wu_r[:], 0.0)

            # K/V weights: both blocks' tiles live through phase A; loads are
            # issued in 4-chunk (1 MB) groups with >=8-chunk lookahead so no
            # burst starves the pT stream
            wkv = {b: (p_wc.tile([128, KL, HEC], BF16, tag=f"wk_{b}", name=f"wk_{b}"),
                       p_wc.tile([128, KL, HEC], BF16, tag=f"wv_{b}", name=f"wv_{b}"))
                   for b in "td"}
            wkv_r = {b: (wk[b].ap().rearrange("(c p) m -> p c m", p=128),
                         wv[b].ap().rearrange("(c p) m -> p c m", p=128))
                     for b in "td"}
            gq = [(b, g) for b in "td" for g in range(KL // 4)]
            gqi = [0]

            def issue_wkv_group(n=1):
                for _ in range(n):
                    if gqi[0] >= len(gq):
                        return
                    gb, g = gq[gqi[0]]
                    gqi[0] += 1
                    hq = slice(g * 4, (g + 1) * 4)
                    nc.scalar.dma_start(wkv[gb][0][:, hq, :], wkv_r[gb][0][:, hq, :])
                    nc.scalar.dma_start(wkv[gb][1][:, hq, :], wkv_r[gb][1][:, hq, :])

            issue_wkv_group(2)

            for b in "td":
                vps = [psA.tile([SCH, 2, HEC], F32, tag=f"vps{i}", name=f"vps{i}")
                       for i in range(4)]
                kps = [psA.tile([128, 512], F32, tag=f"kps{i}", name=f"kps{i}")
                       for i in range(4)]
                if b == "t":
                    # discarded warm-up group in kps[0]'s bank, closed before
                    # the real accumulation group opens
                    for wi in range(16):
                        nc.tensor.matmul(kps[0][:], wu_l[:], wu_r[:],
                                         start=(wi == 0), stop=(wi == 15))
                wk_t, wv_t = wkv[b]
                pt_r = pT[b].ap().rearrange("(c p) s -> c p s", c=KL)
                for kc in range(KL):
                    if kc % 4 == 0:
                        issue_wkv_group(1)
                    if b == "d" and kc == 28:
                        for bb in "td":
                            nc.gpsimd.dma_start(
                                xt0[bb][:],
                                xT[bb].ap().rearrange("(c p) n -> p c n",
                                                      p=128)[:, :, 0:RC])
                    pt_t = p_pt.tile([128, S], BF16, tag="pt")
                    nc.sync.dma_start(pt_t[:], pt_r[kc])
                    wkc = wk_t[:, kc, :]
                    wvc = wv_t[:, kc, :]
                    for si in range(NSC):
                        # one accumulation group per PSUM bank: only the first
                        # half issues start=True (bank-wide clear covers both)
                        nc.tensor.matmul(
                            vps[si // 2][:, si % 2, :],
                            pt_t[:, si * SCH:(si + 1) * SCH],
                            wvc,
                            start=(kc == 0 and si % 2 == 0), stop=False)
                    for mc in range(2):
                        for ncc in range(2):
                            nc.tensor.matmul(
                                kps[mc * 2 + ncc][:, 0:500],
                                wk_t[:, kc, mc * 128:(mc + 1) * 128],
                                pt_t[:, ncc * 500:(ncc + 1) * 500],
                                start=(kc == 0), stop=(kc == KL - 1))
                boff = 0 if b == "t" else HEC
                for si in range(NSC):
                    nc.tensor.matmul(
                        vps[si // 2][:, si % 2, :],
                        ones125[:],
                        bv_sb[:, boff:boff + HEC],
                        start=False, stop=(si % 2 == 1))
                for si in range(NSC):
                    # copy V psum (125, 256) -> [:, si, :, 0:64] viewed as (125, 4, 64)
                    nc.vector.tensor_copy(
                        vx_sb[b][:, si, :, 0:64],
                        vps[si // 2][:, si % 2, :].rearrange("p (h e) -> p h e", h=NH))
                nc.vector.memset(vx_sb[b][:, :, :, 64:65], 1.0)
                for mc in range(2):
                    for ncc in range(2):
                        nc.scalar.activation(
                            kt_sb[b][:, mc, ncc * 500:(ncc + 1) * 500],
                            kps[mc * 2 + ncc][:, 0:500],
                            AF.Identity,
                            bias=bk_sb[:, (0 if b == "t" else 2) + mc:
                                       (0 if b == "t" else 2) + mc + 1])
                if b == "t":
                    # Q weights (needed at phase B start)
                    for bb in "td":
                        nc.scalar.dma_start(
                            wq_t[bb][:],
                            wq[bb].ap().rearrange("(c p) m -> p c m", p=128))

        # ---- phase B: Q projections (per rows-chunk; last chunk deferred) ----
        # Large phase-C weights: allocated after phase A's pools released,
        # streamed on the scalar queue from here; first consumers run at
        # rows-chunk 1 of phase C.
        p_pre2 = pre1ctx.enter_context(tc.tile_pool(name="p_pre2", bufs=1))
        wo_t = p_pre2.tile([128, 4, DLLM], BF16, tag="wo")
        w1_t = p_pre2.tile([128, 2 * KD, D], BF16, tag="w1")
        w2_t = p_pre2.tile([128, KD, 1], BF16, tag="w2")
        gx_t = p_pre2.tile([128, 2 * KD, GR], BF16, tag="gx")
        # engine-driven DMAs occupy their engine: sync is idle from the end
        # of the pT stream until the first out-DMA well into phase C; scalar
        # (Q-proj activations) and gpsimd (xt loads) are not
        nc.sync.dma_start(w1_t[:], w1.ap().rearrange("(c p) m -> p c m", p=128))
        nc.sync.dma_start(gx_t[:], gx.ap().rearrange("(c p) m -> p c m", p=128))
        nc.sync.dma_start(w2_t[:], w2.ap().rearrange("(c p) m -> p c m", p=128))
        nc.sync.dma_start(wo_t[:], wo.ap().rearrange("(c p) n -> p c n", p=128))
        with ExitStack() as bctx:
            p_x = bctx.enter_context(tc.tile_pool(name="p_x", bufs=2))
            psB = bctx.enter_context(tc.tile_pool(name="psB", bufs=2, space="PSUM"))
            for r in range(NRC - 1):
                rsl = slice(r * RC, (r + 1) * RC)
                if r == 0:
                    xt = xt0   # prefetched during phase A
                else:
                    xt = {}
                    for b in "td":
                        xt[b] = p_x.tile([128, KD, RC], BF16, tag=f"x_{b}",
                                         name=f"x_{b}")
                        nc.gpsimd.dma_start(
                            xt[b][:],
                            xT[b].ap().rearrange("(c p) n -> p c n",
                                                 p=128)[:, :, rsl])
                for b in "td":
                    for mc in range(2):
                        qps = psB.tile([128, RC], F32, tag="qps")
                        for kc in range(KD):
                            nc.tensor.matmul(
                                qps[:],
                                wq_t[b][:, kc, mc * 128:(mc + 1) * 128],
                                xt[b][:, kc, :],
                                start=(kc == 0), stop=(kc == KD - 1))
                        nc.scalar.activation(
                            qt_sb[b][:, mc, rsl], qps[:], AF.Identity,
                            bias=bq_sb[:, (0 if b == "t" else 2) + mc:
                                       (0 if b == "t" else 2) + mc + 1])

        # ---- phase C: attention + output projection ----
        # Software pipeline: QK+exp of unit u overlaps PV of unit u-1 at
        # s-chunk granularity; output-projection groups of the previous
        # rows-chunk (plus deferred Q-proj and the gate MLP) interleave
        # between units to keep the PE stream dense.
        with ExitStack() as cctx:
            p_p = cctx.enter_context(tc.tile_pool(name="p_p", bufs=5))
            p_a = cctx.enter_context(tc.tile_pool(name="p_a", bufs=2))
            p_s = cctx.enter_context(tc.tile_pool(name="p_s", bufs=2))
            p_o = cctx.enter_context(tc.tile_pool(name="p_o", bufs=2))
            p_h = cctx.enter_context(tc.tile_pool(name="p_h", bufs=1))
            # psO first so the attention pools (psS/psPV) can be released
            # before the tail, freeing their banks for a deeper tail pool
            psO = cctx.enter_context(tc.tile_pool(name="psO", bufs=2, space="PSUM"))
            attctx = ExitStack()
            psS = attctx.enter_context(tc.tile_pool(name="psS", bufs=1, space="PSUM"))
            psPV = attctx.enter_context(tc.tile_pool(name="psPV", bufs=1, space="PSUM"))

            def emit_qk_exp(b, mc, si, rsl):
                sps2 = psS.tile([SCH, 2, RC], F32, tag="sps", name="sps")
                for hh in range(2):  # row-packed pair, adjacent emission
                    po = hh * 64
                    nc.tensor.matmul(
                        sps2[:, hh, :],
                        kt_sb[b][po:po + 64, mc, si * SCH:(si + 1) * SCH],
                        qt_sb[b][po:po + 64, mc, rsl],
                        start=True, stop=True,
                        tile_position=(po, 0))
                p2 = p_p.tile([SCH, 2, RC], BF16, tag=f"p{si % 2}",
                              name=f"p{si % 2}")
                nc.scalar.activation(p2[:], sps2[:], AF.Exp, scale=0.125)
                return p2

            def emit_pv(aps, b, mc, si, p2):
                for hh in range(2):
                    h = mc * 2 + hh
                    nc.tensor.matmul(
                        aps[hh][:], vx_sb[b][:, si, h, :], p2[:, hh, :],
                        start=(si == 0), stop=(si == NSC - 1))

            def emit_norm(aps, b, mc, a2, rsl):
                for hh in range(2):
                    den1 = p_s.tile([1, RC], F32, tag="den1")
                    nc.vector.tensor_copy(den1[:], aps[hh][64:65, :])
                    rec1 = p_s.tile([1, RC], F32, tag="rec1")
                    nc.vector.reciprocal_approx_fast(rec1[:], den1[:])
                    sct64 = p_s.tile([64, RC], F32, tag="sct64")
                    nc.gpsimd.partition_broadcast(sct64[:], rec1[:])
                    nc.vector.tensor_mul(
                        a2[b][mc][hh * 64:hh * 64 + 64, :],
                        aps[hh][0:64, :], sct64[:])

            # deferred phase-B work for the last rows-chunk, plus the gate
            # MLP for this core's 512-row slice — PE filler with no
            # PSUM->SBUF copy load, spread across the rows-chunks
            ht = p_h.tile([128, KD, GR], BF16, tag="ht")
            bstate = {}

            def emit_b3_load(rB):
                rslB = slice(rB * RC, (rB + 1) * RC)
                xtB = {}
                for b in "td":
                    # reuses the rc0-prefetch pool (its phase-B reads are done)
                    xtB[b] = p_x0.tile([128, KD, RC], BF16, tag=f"x0_{b}",
                                       name=f"xB_{b}")
                    nc.gpsimd.dma_start(
                        xtB[b][:],
                        xT[b].ap().rearrange("(c p) n -> p c n", p=128)[:, :, rslB])
                bstate[rB] = xtB

            def emit_b3_q(rB, bb, mcq):
                rslB = slice(rB * RC, (rB + 1) * RC)
                xtB = bstate[rB]
                qps = psO.tile([128, RC], F32, tag="ops", name="qpsB")
                for kc in range(KD):
                    nc.tensor.matmul(
                        qps[:],
                        wq_t[bb][:, kc, mcq * 128:(mcq + 1) * 128],
                        xtB[bb][:, kc, :],
                        start=(kc == 0), stop=(kc == KD - 1))
                nc.scalar.activation(
                    qt_sb[bb][:, mcq, rslB], qps[:], AF.Identity,
                    bias=bq_sb[:, (0 if bb == "t" else 2) + mcq:
                               (0 if bb == "t" else 2) + mcq + 1])

            def emit_gate_hidden(mc):
                hps = psO.tile([128, GR], F32, tag="ops", name="hpsG")
                for kc in range(2 * KD):
                    nc.tensor.matmul(
                        hps[:],
                        w1_t[:, kc, mc * 128:(mc + 1) * 128],
                        gx_t[:, kc, :],
                        start=(kc == 0), stop=(kc == 2 * KD - 1))
                nc.scalar.activation(
                    ht[:, mc, :], hps[:], AF.Relu, bias=gb1_sb[:, mc:mc + 1])

            def emit_gate_logit():
                lpst = psO.tile([128, GR], F32, tag="ops", name="lpsG")
                for mc in range(KD):
                    nc.tensor.matmul(
                        lpst[0:1, :], w2_t[:, mc, :], ht[:, mc, :],
                        start=(mc == 0), stop=(mc == KD - 1))
                gate_sb = pers.tile([1, GR], F32, tag="gate")
                nc.scalar.activation(gate_sb[:], lpst[0:1, :], AF.Sigmoid,
                                     bias=gb2_sb[:])
                nc.sync.dma_start(gate_dr.ap(), gate_sb[:])

            gtasks = [lambda: emit_b3_load(NRC - 1)]
            for bb in "td":
                for mcq in range(2):
                    gtasks.append(
                        lambda bb=bb, mcq=mcq: emit_b3_q(NRC - 1, bb, mcq))
            for mc in range(KD):
                gtasks.append(lambda mc=mc: emit_gate_hidden(mc))
            gtasks.append(emit_gate_logit)
            # rows-chunk r consumes gtasks[gshare[r]:gshare[r+1]] as extra
            # filler: rc0 the deferred Q-proj (before rc3 needs it), then the
            # gate spread over rc1-rc3
            gshare = [0, 9, 11, 13, 14]

            # copy-engine rotation for output-projection PSUM->SBUF moves:
            # GpSimd cannot read PSUM; Act also carries the exp stream
            cp_engines = [nc.vector, nc.vector, nc.scalar]

            def make_outproj_tasks(a2p, r_prev, pool=None, rot=None, tag="ops"):
                """One task per PSUM group; osb assembled per (bb, rb, half)."""
                state = {}
                cnt = [0]
                pool_ = pool or psO
                rot_ = rot or cp_engines

                def group(bb, rb, ncc):
                    row0 = r_prev * RC + rb * 128
                    half, nh = divmod(ncc, 4)
                    if nh == 0:
                        state[(bb, rb, half)] = p_o.tile(
                            [128, DLLM // 2], BF16, tag=f"osb_{bb}", name="osb")
                    osb = state[(bb, rb, half)]
                    nsl = slice(ncc * 512, (ncc + 1) * 512)
                    ops = pool_.tile([128, 512], F32, tag=tag, name="ops")
                    ko = 0 if bb == "t" else 2
                    for mcc in range(2):
                        nc.tensor.matmul(
                            ops[:], a2p[bb][mcc][:, rb * 128:(rb + 1) * 128],
                            wo_t[:, ko + mcc, nsl],
                            start=(mcc == 0), stop=(mcc == 1))
                    eng = rot_[cnt[0] % len(rot_)]
                    cnt[0] += 1
                    if eng is nc.scalar:
                        eng.activation(osb[:, nh * 512:(nh + 1) * 512], ops[:],
                                       AF.Identity)
                    else:
                        eng.tensor_copy(osb[:, nh * 512:(nh + 1) * 512], ops[:])
                    if nh == 3:
                        nc.sync.dma_start(
                            out_dr[bb].ap()[row0:row0 + 128,
                                            half * 2048:(half + 1) * 2048],
                            osb[:])

                return [(lambda bb=bb, rb=rb, ncc=ncc: group(bb, rb, ncc))
                        for rb in range(4) for ncc in range(8) for bb in "td"]

            pending = None   # (a2 dict, r) awaiting output projection
            for r in range(NRC):
                rsl = slice(r * RC, (r + 1) * RC)
                a2 = {b: [p_a.tile([128, RC], BF16, tag=f"a2_{b}{mc}",
                                   name=f"a2_{b}{mc}")
                          for mc in range(2)] for b in "td"}
                units = [(b, mc) for b in "td" for mc in range(2)]
                # filler: out-proj groups of r-1 plus this rows-chunk's share
                # of the deferred Q/gate queue
                tasks = list(gtasks[gshare[r]:gshare[r + 1]])
                if pending is not None:
                    tasks = make_outproj_tasks(pending[0], pending[1]) + tasks
                ti = 0
                nslots = len(units) * NSC
                prev = None   # (aps, b, mc, p2dict)
                slot = 0
                for u, (b, mc) in enumerate(units):
                    aps = [psPV.tile([65, RC], F32, tag=f"aps{mc}{hh}",
                                     name=f"aps{mc}{hh}") for hh in range(2)]
                    p2buf = {}
                    for si in range(NSC):
                        if prev is not None:
                            paps, pb, pmc, pp2 = prev
                            emit_pv(paps, pb, pmc, si, pp2[si])
                        p2buf[si] = emit_qk_exp(b, mc, si, rsl)
                        # paced filler: one task per due slot
                        slot += 1
                        due = (slot * len(tasks)) // nslots
                        while ti < due:
                            tasks[ti]()
                            ti += 1
                    if prev is not None:
                        emit_norm(prev[0], prev[1], prev[2], a2, rsl)
                    prev = (aps, b, mc, p2buf)
                # drain last unit of this rows-chunk
                paps, pb, pmc, pp2 = prev
                for si in range(NSC):
                    emit_pv(paps, pb, pmc, si, pp2[si])
                emit_norm(paps, pb, pmc, a2, rsl)
                while ti < len(tasks):
                    tasks[ti]()
                    ti += 1
                pending = (a2, r)
            # tail: attention pools released; a 6-bank pool and an even
            # DVE/Act rotation keep the final out-projection copy-unbound
            attctx.close()
            psT = cctx.enter_context(tc.tile_pool(name="psT", bufs=6,
                                                  space="PSUM"))
            for task in make_outproj_tasks(pending[0], pending[1], pool=psT,
                                           rot=[nc.vector, nc.scalar],
                                           tag="ops2"):
                task()
        pre1ctx.close()

    nc.compile()
    return nc


def _prep_inputs(inputs):
    """Host-side shard + transpose. Returns in_maps for 8 cores."""
    f32 = np.float32
    t = {k: np.asarray(v) for k, v in inputs.items()}
    x_full = {"t": t["trend_emb"].reshape(B * L, D).astype(f32),
              "d": t["detail_emb"].reshape(B * L, D).astype(f32)}
    pT_full = {"t": np.ascontiguousarray(t["trend_proto"].astype(f32).T).astype(bf16),
               "d": np.ascontiguousarray(t["detail_proto"].astype(f32).T).astype(bf16)}
    cat = np.concatenate([x_full["t"], x_full["d"]], axis=1)  # (B*L, 2D)
    w1_bf = t["g_W1"].astype(bf16)
    w2_bf = t["g_W2"].astype(bf16)
    gb1_f = np.ascontiguousarray(t["g_b1"].astype(f32).reshape(KD, 128).T)
    gb2_f = t["g_b2"].astype(f32).reshape(1, 1)
    W = {("q", "t"): t["t_Wq"], ("q", "d"): t["d_Wq"],
         ("k", "t"): t["t_Wk"], ("k", "d"): t["d_Wk"],
         ("v", "t"): t["t_Wv"], ("v", "d"): t["d_Wv"],
         ("o", "t"): t["t_Wo"], ("o", "d"): t["d_Wo"]}
    bias = {("q", "t"): t["t_bq"], ("q", "d"): t["d_bq"],
            ("k", "t"): t["t_bk"], ("k", "d"): t["d_bk"],
            ("v", "t"): t["t_bv"], ("v", "d"): t["d_bv"],
            ("o", "t"): t["t_bo"], ("o", "d"): t["d_bo"]}

    in_maps = []
    for core in range(8):
        rg, hg = divmod(core, HG)
        rows = slice(rg * R, (rg + 1) * R)
        hsl = slice(hg * HEC, (hg + 1) * HEC)
        gsl = slice(core * GR, (core + 1) * GR)
        m = {}
        for b in "td":
            m[f"xT_{b}"] = np.ascontiguousarray(x_full[b][rows].T).astype(bf16)
            m[f"pT_{b}"] = pT_full[b]
            m[f"wq_{b}"] = np.ascontiguousarray(W[("q", b)][:, hsl]).astype(bf16)
            m[f"wk_{b}"] = np.ascontiguousarray(W[("k", b)][:, hsl]).astype(bf16)
            m[f"wv_{b}"] = np.ascontiguousarray(W[("v", b)][:, hsl]).astype(bf16)
        m["wo"] = np.vstack([W[("o", "t")][hsl, :], W[("o", "d")][hsl, :]]).astype(bf16)
        m["w1"] = w1_bf
        m["w2"] = w2_bf
        m["gxT"] = np.ascontiguousarray(cat[gsl].T).astype(bf16)
        m["bq2"] = np.stack([bias[("q", "t")][hsl][0:128], bias[("q", "t")][hsl][128:256],
                             bias[("q", "d")][hsl][0:128], bias[("q", "d")][hsl][128:256]],
                            axis=1).astype(f32)
        m["bk2"] = np.stack([bias[("k", "t")][hsl][0:128], bias[("k", "t")][hsl][128:256],
                             bias[("k", "d")][hsl][0:128], bias[("k", "d")][hsl][128:256]],
                            axis=1).astype(f32)
        m["bv"] = np.concatenate([bias[("v", "t")][hsl],
                                  bias[("v", "d")][hsl]])[None, :].astype(bf16)
        m["gb1"] = gb1_f
        m["gb2"] = gb2_f
        in_maps.append(m)
    bo = (np.asarray(bias[("o", "t")], f32), np.asarray(bias[("o", "d")], f32))
    return in_maps, bo


def kernel(**inputs):
    global LAST_RESULTS
    import os
    from concourse.bass_utils import run_bass_kernel_spmd

    in_maps, (bo_t, bo_d) = _prep_inputs(inputs)
    if "nc" not in _CACHE:
        _CACHE["nc"] = _build()
    nc = _CACHE["nc"]

    trace = bool(os.environ.get("KERNEL_TRACE"))
    res = run_bass_kernel_spmd(
        nc, in_maps, list(range(8)),
        trace=trace, trace_cores=list(range(8)) if trace else None)
    LAST_RESULTS = res

    f32 = np.float32
    g = np.concatenate([np.asarray(res.results[c]["gate"]).reshape(GR)
                        for c in range(8)]).astype(f32)[:, None]   # (B*L, 1)
    out = np.empty((RG, R, DLLM), f32)
    for rg in range(RG):
        Pt = res.results[rg * HG]["out_t"].astype(f32)
        Pd = res.results[rg * HG]["out_d"].astype(f32)
        for hg in range(1, HG):
            Pt += res.results[rg * HG + hg]["out_t"].astype(f32)
            Pd += res.results[rg * HG + hg]["out_d"].astype(f32)
        grg = g[rg * R:(rg + 1) * R]
        out[rg] = grg * Pt + (1.0 - grg) * Pd
        if np.any(bo_t) or np.any(bo_d):
            out[rg] += grg * bo_t[None, :] + (1.0 - grg) * bo_d[None, :]
    return out.reshape(B, L, DLLM)


# revision 44
# speedup vs baseline: 1.2456x; 1.0119x over previous
"""DualReprogrammingLayer Trainium2 kernel.

Sharding: 2 row-groups (B*L split in halves) x 4 head-groups (4 heads each).
Each core computes, for its 2048 rows and 4 heads (per block in {trend, detail}):
  KT = (Wk.T @ protoT)           (heads-slice, S)
  V  = (protoT.T @ Wv) + bv      (S, heads-slice)
  qT = (Wq.T @ xT) + bq          (heads-slice, rows)
  scoresT = KT_h @ qT_h          (S, rows) per head   [2-head row-packed]
  P  = exp(scoresT / 8)
  A_ext = [V_h | ones].T @ P     (64+64, rows)        [rows 64:128 = denom]
  A_n = A / denom                                      [no gate on device]
  out_b_partial = A_b.T-chunks @ Wo_b  (2048, 4096) bf16, b in {t, d}
Gate MLP is row-sharded 8 ways instead of computed redundantly per core:
core c computes gate = sigmoid(relu(gx@W1)@W2) for global rows
[512c, 512c+512) only (gx is a per-core input slice).
Host: per row-group, Pt = sum_hg out_t, Pd = sum_hg out_d;
out = g*Pt + (1-g)*Pd (+ g*bo_t + (1-g)*bo_d), g assembled from 8 slices.
"""
import sys
sys.path.insert(0, '/opt/trn_rl_repo')
from contextlib import ExitStack

import numpy as np
import ml_dtypes

import concourse.bass as bass
import concourse.tile as tile
from concourse import bacc, mybir

F32 = mybir.dt.float32
BF16 = mybir.dt.bfloat16
AF = mybir.ActivationFunctionType
bf16 = ml_dtypes.bfloat16

B, L, D, S, DLLM, H, E = 4, 1024, 1024, 1000, 4096, 16, 64
RG, HG = 2, 4                 # row-groups x head-groups = 8 cores
R = (B * L) // RG             # 2048 rows per core
NH = H // HG                  # 4 heads per core
HEC = NH * E                  # 256
GR = (B * L) // 8             # 512 gate rows per core
SCH, NSC = 125, 8             # S = 8 chunks of 125
RC, NRC = 512, 4              # rows = 4 chunks of 512
KD = D // 128                 # 8 k-chunks for d_model
KL = DLLM // 128              # 32 k-chunks for d_llm

_CACHE = {}
LAST_RESULTS = None           # set by kernel(): BassKernelResults


def _build():
    nc = bacc.Bacc("TRN2", target_bir_lowering=False, debug=False)

    def din(name, shape, dt):
        return nc.dram_tensor(name, list(shape), dt, kind="ExternalInput")

    xT = {b: din(f"xT_{b}", (D, R), BF16) for b in "td"}
    pT = {b: din(f"pT_{b}", (DLLM, S), BF16) for b in "td"}
    wq = {b: din(f"wq_{b}", (D, HEC), BF16) for b in "td"}
    wk = {b: din(f"wk_{b}", (DLLM, HEC), BF16) for b in "td"}
    wv = {b: din(f"wv_{b}", (DLLM, HEC), BF16) for b in "td"}
    wo = din("wo", (2 * HEC, DLLM), BF16)            # [t rows | d rows]
    w1 = din("w1", (2 * D, D), BF16)
    w2 = din("w2", (D, 1), BF16)
    gx = din("gxT", (2 * D, GR), BF16)               # cat(x_t,x_d)[gate rows].T
    bq2 = din("bq2", (128, 4), F32)                  # cols: t-mc0, t-mc1, d-mc0, d-mc1
    bk2 = din("bk2", (128, 4), F32)
    bvv = din("bv", (1, 2 * HEC), BF16)              # [t 256 | d 256]
    gb1 = din("gb1", (128, KD), F32)
    gb2 = din("gb2", (1, 1), F32)
    out_dr = {b: nc.dram_tensor(f"out_{b}", [R, DLLM], BF16,
                                kind="ExternalOutput") for b in "td"}
    gate_dr = nc.dram_tensor("gate", [1, GR], F32, kind="ExternalOutput")

    with tile.TileContext(nc) as tc, ExitStack() as ctx:
        # ---- persistent pools (live across phases) ----
        pers = ctx.enter_context(tc.tile_pool(name="pers", bufs=1))
        kt_sb = {}    # block -> tile (128, 2, S) bf16 : HE chunk mc at [:, mc, :]
        vx_sb = {}    # block -> tile (125, NSC, NH, 65) bf16 : [V_h | ones]
        qt_sb = {}    # block -> tile (128, 2, R) bf16
        for b in "td":
            kt_sb[b] = pers.tile([128, 2, S], BF16, tag=f"kt_{b}", name=f"kt_{b}")
            vx_sb[b] = pers.tile([SCH, NSC, NH, 65], BF16, tag=f"vx_{b}", name=f"vx_{b}")
            qt_sb[b] = pers.tile([128, 2, R], BF16, tag=f"qt_{b}", name=f"qt_{b}")
        ones125 = pers.tile([1, SCH], BF16, tag="ones125")
        nc.vector.memset(ones125[:], 1.0)
        bq_sb = pers.tile([128, 4], F32, tag="bq")
        nc.gpsimd.dma_start(bq_sb[:], bq2.ap())
        bk_sb = pers.tile([128, 4], F32, tag="bk")
        nc.gpsimd.dma_start(bk_sb[:], bk2.ap())
        bv_sb = pers.tile([1, 2 * HEC], BF16, tag="bv")
        nc.gpsimd.dma_start(bv_sb[:], bvv.ap())
        gb1_sb = pers.tile([128, KD], F32, tag="gb1")
        nc.gpsimd.dma_start(gb1_sb[:], gb1.ap())
        gb2_sb = pers.tile([1, 1], F32, tag="gb2")
        nc.gpsimd.dma_start(gb2_sb[:], gb2.ap())

        # ---- prefetch pools. p_pre1 (wq) is loaded during phase A; the
        # large phase-C weights (wo/w1/gx/w2) are allocated and loaded at
        # phase B start to keep phase-A SBUF small. Both live through C.
        pre1ctx = ExitStack()
        p_pre1 = pre1ctx.enter_context(tc.tile_pool(name="p_pre1", bufs=1))
        wq_t = {}
        for b in "td":
            wq_t[b] = p_pre1.tile([128, KD, HEC], BF16, tag=f"wq_{b}", name=f"wq_{b}")
        # rc0's x tiles, prefetched during phase A so phase B starts hot;
        # the pool is reused for the deferred last rows-chunk load in C
        p_x0 = pre1ctx.enter_context(tc.tile_pool(name="p_x0", bufs=1))
        xt0 = {b: p_x0.tile([128, KD, RC], BF16, tag=f"x0_{b}", name=f"x0_{b}")
               for b in "td"}

        # ---- phase A: K/V projections (proto and weights streamed per k-chunk) ----
        with ExitStack() as actx:
            p_pt = actx.enter_context(tc.tile_pool(name="p_pt", bufs=12))
            p_wc = actx.enter_context(tc.tile_pool(name="p_wc", bufs=1))
            psA = actx.enter_context(tc.tile_pool(name="psA", bufs=1, space="PSUM"))
            # PE warm-up: ~8us of dummy matmuls on memset tiles so the HAM
            # clock-gate reaches 8/8 before the first real matmul, overlapping
            # the initial DMA ramp. Results are discarded.
            wu_l = p_wc.tile([64, 128], BF16, tag="wu_l")
            nc.vector.memset(wu_l[:], 0.0)
            wu_r = p_wc.tile([64, 512], BF16, tag="wu_r")
            nc.vector.memset(wu_r[:], 0.0)

            # K/V weights: both blocks' tiles live through phase A; loads are
            # issued in 4-chunk (1 MB) groups with >=8-chunk lookahead so no
            # burst starves the pT stream
            wkv = {b: (p_wc.tile([128, KL, HEC], BF16, tag=f"wk_{b}", name=f"wk_{b}"),
                       p_wc.tile([128, KL, HEC], BF16, tag=f"wv_{b}", name=f"wv_{b}"))
                   for b in "td"}
            wkv_r = {b: (wk[b].ap().rearrange("(c p) m -> p c m", p=128),
                         wv[b].ap().rearrange("(c p) m -> p c m", p=128))
                     for b in "td"}
            gq = [(b, g) for b in "td" for g in range(KL // 4)]
            gqi = [0]

            def issue_wkv_group(n=1):
                for _ in range(n):
                    if gqi[0] >= len(gq):
                        return
                    gb, g = gq[gqi[0]]
                    gqi[0] += 1
                    hq = slice(g * 4, (g + 1) * 4)
                    nc.scalar.dma_start(wkv[gb][0][:, hq, :], wkv_r[gb][0][:, hq, :])
                    nc.scalar.dma_start(wkv[gb][1][:, hq, :], wkv_r[gb][1][:, hq, :])

            issue_wkv_group(2)

            for b in "td":
                vps = [psA.tile([SCH, 2, HEC], F32, tag=f"vps{i}", name=f"vps{i}")
                       for i in range(4)]
                kps = [psA.tile([128, 512], F32, tag=f"kps{i}", name=f"kps{i}")
                       for i in range(4)]
                if b == "t":
                    # discarded warm-up group in kps[0]'s bank, closed before
                    # the real accumulation group opens
                    for wi in range(16):
                        nc.tensor.matmul(kps[0][:], wu_l[:], wu_r[:],
                                         start=(wi == 0), stop=(wi == 15))
                wk_t, wv_t = wkv[b]
                pt_r = pT[b].ap().rearrange("(c p) s -> c p s", c=KL)
                for kc in range(KL):
                    if kc % 4 == 0:
                        issue_wkv_group(1)
                    if b == "d" and kc == 28:
                        for bb in "td":
                            nc.gpsimd.dma_start(
                                xt0[bb][:],
                                xT[bb].ap().rearrange("(c p) n -> p c n",
                                                      p=128)[:, :, 0:RC])
                    pt_t = p_pt.tile([128, S], BF16, tag="pt")
                    # alternate the pT stream across two DMA queues: halves
                    # per-queue latency jitter that was dropping the PE out
                    # of its fast p-state
                    (nc.sync if kc % 2 == 0 else nc.scalar).dma_start(
                        pt_t[:], pt_r[kc])
                    wkc = wk_t[:, kc, :]
                    wvc = wv_t[:, kc, :]
                    for si in range(NSC):
                        # one accumulation group per PSUM bank: only the first
                        # half issues start=True (bank-wide clear covers both)
                        nc.tensor.matmul(
                            vps[si // 2][:, si % 2, :],
                            pt_t[:, si * SCH:(si + 1) * SCH],
                            wvc,
                            start=(kc == 0 and si % 2 == 0), stop=False)
                    for mc in range(2):
                        for ncc in range(2):
                            nc.tensor.matmul(
                                kps[mc * 2 + ncc][:, 0:500],
                                wk_t[:, kc, mc * 128:(mc + 1) * 128],
                                pt_t[:, ncc * 500:(ncc + 1) * 500],
                                start=(kc == 0), stop=(kc == KL - 1))
                boff = 0 if b == "t" else HEC
                for si in range(NSC):
                    nc.tensor.matmul(
                        vps[si // 2][:, si % 2, :],
                        ones125[:],
                        bv_sb[:, boff:boff + HEC],
                        start=False, stop=(si % 2 == 1))
                for si in range(NSC):
                    # copy V psum (125, 256) -> [:, si, :, 0:64] viewed as (125, 4, 64)
                    nc.vector.tensor_copy(
                        vx_sb[b][:, si, :, 0:64],
                        vps[si // 2][:, si % 2, :].rearrange("p (h e) -> p h e", h=NH))
                nc.vector.memset(vx_sb[b][:, :, :, 64:65], 1.0)
                for mc in range(2):
                    for ncc in range(2):
                        nc.scalar.activation(
                            kt_sb[b][:, mc, ncc * 500:(ncc + 1) * 500],
                            kps[mc * 2 + ncc][:, 0:500],
                            AF.Identity,
                            bias=bk_sb[:, (0 if b == "t" else 2) + mc:
                                       (0 if b == "t" else 2) + mc + 1])
                if b == "t":
                    # Q weights (needed at phase B start)
                    for bb in "td":
                        nc.scalar.dma_start(
                            wq_t[bb][:],
                            wq[bb].ap().rearrange("(c p) m -> p c m", p=128))

        # ---- phase B: Q projections (per rows-chunk; last chunk deferred) ----
        # Large phase-C weights: allocated after phase A's pools released,
        # streamed on the scalar queue from here; first consumers run at
        # rows-chunk 1 of phase C.
        p_pre2 = pre1ctx.enter_context(tc.tile_pool(name="p_pre2", bufs=1))
        wo_t = p_pre2.tile([128, 4, DLLM], BF16, tag="wo")
        w1_t = p_pre2.tile([128, 2 * KD, D], BF16, tag="w1")
        w2_t = p_pre2.tile([128, KD, 1], BF16, tag="w2")
        gx_t = p_pre2.tile([128, 2 * KD, GR], BF16, tag="gx")
        # engine-driven DMAs occupy their engine: sync is idle from the end
        # of the pT stream until the first out-DMA well into phase C; scalar
        # (Q-proj activations) and gpsimd (xt loads) are not
        nc.sync.dma_start(w1_t[:], w1.ap().rearrange("(c p) m -> p c m", p=128))
        nc.sync.dma_start(gx_t[:], gx.ap().rearrange("(c p) m -> p c m", p=128))
        nc.sync.dma_start(w2_t[:], w2.ap().rearrange("(c p) m -> p c m", p=128))
        nc.sync.dma_start(wo_t[:], wo.ap().rearrange("(c p) n -> p c n", p=128))
        with ExitStack() as bctx:
            p_x = bctx.enter_context(tc.tile_pool(name="p_x", bufs=2))
            psB = bctx.enter_context(tc.tile_pool(name="psB", bufs=2, space="PSUM"))
            for r in range(NRC - 1):
                rsl = slice(r * RC, (r + 1) * RC)
                if r == 0:
                    xt = xt0   # prefetched during phase A
                else:
                    xt = {}
                    for b in "td":
                        xt[b] = p_x.tile([128, KD, RC], BF16, tag=f"x_{b}",
                                         name=f"x_{b}")
                        nc.gpsimd.dma_start(
                            xt[b][:],
                            xT[b].ap().rearrange("(c p) n -> p c n",
                                                 p=128)[:, :, rsl])
                for b in "td":
                    for mc in range(2):
                        qps = psB.tile([128, RC], F32, tag="qps")
                        for kc in range(KD):
                            nc.tensor.matmul(
                                qps[:],
                                wq_t[b][:, kc, mc * 128:(mc + 1) * 128],
                                xt[b][:, kc, :],
                                start=(kc == 0), stop=(kc == KD - 1))
                        nc.scalar.activation(
                            qt_sb[b][:, mc, rsl], qps[:], AF.Identity,
                            bias=bq_sb[:, (0 if b == "t" else 2) + mc:
                                       (0 if b == "t" else 2) + mc + 1])

        # ---- phase C: attention + output projection ----
        # Software pipeline: QK+exp of unit u overlaps PV of unit u-1 at
        # s-chunk granularity; output-projection groups of the previous
        # rows-chunk (plus deferred Q-proj and the gate MLP) interleave
        # between units to keep the PE stream dense.
        with ExitStack() as cctx:
            p_p = cctx.enter_context(tc.tile_pool(name="p_p", bufs=5))
            p_a = cctx.enter_context(tc.tile_pool(name="p_a", bufs=2))
            p_s = cctx.enter_context(tc.tile_pool(name="p_s", bufs=2))
            p_o = cctx.enter_context(tc.tile_pool(name="p_o", bufs=2))
            p_h = cctx.enter_context(tc.tile_pool(name="p_h", bufs=1))
            # psO first so the attention pools (psS/psPV) can be released
            # before the tail, freeing their banks for a deeper tail pool
            psO = cctx.enter_context(tc.tile_pool(name="psO", bufs=2, space="PSUM"))
            attctx = ExitStack()
            psS = attctx.enter_context(tc.tile_pool(name="psS", bufs=1, space="PSUM"))
            psPV = attctx.enter_context(tc.tile_pool(name="psPV", bufs=1, space="PSUM"))

            def emit_qk_exp(b, mc, si, rsl):
                sps2 = psS.tile([SCH, 2, RC], F32, tag="sps", name="sps")
                for hh in range(2):  # row-packed pair, adjacent emission
                    po = hh * 64
                    nc.tensor.matmul(
                        sps2[:, hh, :],
                        kt_sb[b][po:po + 64, mc, si * SCH:(si + 1) * SCH],
                        qt_sb[b][po:po + 64, mc, rsl],
                        start=True, stop=True,
                        tile_position=(po, 0))
                p2 = p_p.tile([SCH, 2, RC], BF16, tag=f"p{si % 2}",
                              name=f"p{si % 2}")
                nc.scalar.activation(p2[:], sps2[:], AF.Exp, scale=0.125)
                return p2

            def emit_pv(aps, b, mc, si, p2):
                for hh in range(2):
                    h = mc * 2 + hh
                    nc.tensor.matmul(
                        aps[hh][:], vx_sb[b][:, si, h, :], p2[:, hh, :],
                        start=(si == 0), stop=(si == NSC - 1))

            def emit_norm(aps, b, mc, a2, rsl):
                for hh in range(2):
                    den1 = p_s.tile([1, RC], F32, tag="den1")
                    nc.vector.tensor_copy(den1[:], aps[hh][64:65, :])
                    rec1 = p_s.tile([1, RC], F32, tag="rec1")
                    nc.vector.reciprocal_approx_fast(rec1[:], den1[:])
                    sct64 = p_s.tile([64, RC], F32, tag="sct64")
                    nc.gpsimd.partition_broadcast(sct64[:], rec1[:])
                    nc.vector.tensor_mul(
                        a2[b][mc][hh * 64:hh * 64 + 64, :],
                        aps[hh][0:64, :], sct64[:])

            # deferred phase-B work for the last rows-chunk, plus the gate
            # MLP for this core's 512-row slice — PE filler with no
            # PSUM->SBUF copy load, spread across the rows-chunks
            ht = p_h.tile([128, KD, GR], BF16, tag="ht")
            bstate = {}

            def emit_b3_load(rB):
                rslB = slice(rB * RC, (rB + 1) * RC)
                xtB = {}
                for b in "td":
                    # reuses the rc0-prefetch pool (its phase-B reads are done)
                    xtB[b] = p_x0.tile([128, KD, RC], BF16, tag=f"x0_{b}",
                                       name=f"xB_{b}")
                    nc.gpsimd.dma_start(
                        xtB[b][:],
                        xT[b].ap().rearrange("(c p) n -> p c n", p=128)[:, :, rslB])
                bstate[rB] = xtB

            def emit_b3_q(rB, bb, mcq):
                rslB = slice(rB * RC, (rB + 1) * RC)
                xtB = bstate[rB]
                qps = psO.tile([128, RC], F32, tag="ops", name="qpsB")
                for kc in range(KD):
                    nc.tensor.matmul(
                        qps[:],
                        wq_t[bb][:, kc, mcq * 128:(mcq + 1) * 128],
                        xtB[bb][:, kc, :],
                        start=(kc == 0), stop=(kc == KD - 1))
                nc.scalar.activation(
                    qt_sb[bb][:, mcq, rslB], qps[:], AF.Identity,
                    bias=bq_sb[:, (0 if bb == "t" else 2) + mcq:
                               (0 if bb == "t" else 2) + mcq + 1])

            def emit_gate_hidden(mc):
                hps = psO.tile([128, GR], F32, tag="ops", name="hpsG")
                for kc in range(2 * KD):
                    nc.tensor.matmul(
                        hps[:],
                        w1_t[:, kc, mc * 128:(mc + 1) * 128],
                        gx_t[:, kc, :],
                        start=(kc == 0), stop=(kc == 2 * KD - 1))
                nc.scalar.activation(
                    ht[:, mc, :], hps[:], AF.Relu, bias=gb1_sb[:, mc:mc + 1])

            def emit_gate_logit():
                lpst = psO.tile([128, GR], F32, tag="ops", name="lpsG")
                for mc in range(KD):
                    nc.tensor.matmul(
                        lpst[0:1, :], w2_t[:, mc, :], ht[:, mc, :],
                        start=(mc == 0), stop=(mc == KD - 1))
                gate_sb = pers.tile([1, GR], F32, tag="gate")
                nc.scalar.activation(gate_sb[:], lpst[0:1, :], AF.Sigmoid,
                                     bias=gb2_sb[:])
                nc.sync.dma_start(gate_dr.ap(), gate_sb[:])

            gtasks = [lambda: emit_b3_load(NRC - 1)]
            for bb in "td":
                for mcq in range(2):
                    gtasks.append(
                        lambda bb=bb, mcq=mcq: emit_b3_q(NRC - 1, bb, mcq))
            for mc in range(KD):
                gtasks.append(lambda mc=mc: emit_gate_hidden(mc))
            gtasks.append(emit_gate_logit)
            # rows-chunk r consumes gtasks[gshare[r]:gshare[r+1]] as extra
            # filler: rc0 the deferred Q-proj (before rc3 needs it), then the
            # gate spread over rc1-rc3
            gshare = [0, 9, 11, 13, 14]

            # copy-engine rotation for output-projection PSUM->SBUF moves:
            # GpSimd cannot read PSUM; Act also carries the exp stream
            cp_engines = [nc.vector, nc.vector, nc.scalar]

            def make_outproj_tasks(a2p, r_prev, pool=None, rot=None, tag="ops"):
                """One task per PSUM group; osb assembled per (bb, rb, half)."""
                state = {}
                cnt = [0]
                pool_ = pool or psO
                rot_ = rot or cp_engines

                def group(bb, rb, ncc):
                    row0 = r_prev * RC + rb * 128
                    half, nh = divmod(ncc, 4)
                    if nh == 0:
                        state[(bb, rb, half)] = p_o.tile(
                            [128, DLLM // 2], BF16, tag=f"osb_{bb}", name="osb")
                    osb = state[(bb, rb, half)]
                    nsl = slice(ncc * 512, (ncc + 1) * 512)
                    ops = pool_.tile([128, 512], F32, tag=tag, name="ops")
                    ko = 0 if bb == "t" else 2
                    for mcc in range(2):
                        nc.tensor.matmul(
                            ops[:], a2p[bb][mcc][:, rb * 128:(rb + 1) * 128],
                            wo_t[:, ko + mcc, nsl],
                            start=(mcc == 0), stop=(mcc == 1))
                    eng = rot_[cnt[0] % len(rot_)]
                    cnt[0] += 1
                    if eng is nc.scalar:
                        eng.activation(osb[:, nh * 512:(nh + 1) * 512], ops[:],
                                       AF.Identity)
                    else:
                        eng.tensor_copy(osb[:, nh * 512:(nh + 1) * 512], ops[:])
                    if nh == 3:
                        nc.sync.dma_start(
                            out_dr[bb].ap()[row0:row0 + 128,
                                            half * 2048:(half + 1) * 2048],
                            osb[:])

                return [(lambda bb=bb, rb=rb, ncc=ncc: group(bb, rb, ncc))
                        for rb in range(4) for ncc in range(8) for bb in "td"]

            pending = None   # (a2 dict, r) awaiting output projection
            for r in range(NRC):
                rsl = slice(r * RC, (r + 1) * RC)
                a2 = {b: [p_a.tile([128, RC], BF16, tag=f"a2_{b}{mc}",
                                   name=f"a2_{b}{mc}")
                          for mc in range(2)] for b in "td"}
                units = [(b, mc) for b in "td" for mc in range(2)]
                # filler: out-proj groups of r-1 plus this rows-chunk's share
                # of the deferred Q/gate queue
                tasks = list(gtasks[gshare[r]:gshare[r + 1]])
                if pending is not None:
                    tasks = make_outproj_tasks(pending[0], pending[1]) + tasks
                ti = 0
                nslots = len(units) * NSC
                prev = None   # (aps, b, mc, p2dict)
                slot = 0
                for u, (b, mc) in enumerate(units):
                    aps = [psPV.tile([65, RC], F32, tag=f"aps{mc}{hh}",
                                     name=f"aps{mc}{hh}") for hh in range(2)]
                    p2buf = {}
                    for si in range(NSC):
                        if prev is not None:
                            paps, pb, pmc, pp2 = prev
                            emit_pv(paps, pb, pmc, si, pp2[si])
                        p2buf[si] = emit_qk_exp(b, mc, si, rsl)
                        # paced filler: one task per due slot
                        slot += 1
                        due = (slot * len(tasks)) // nslots
                        while ti < due:
                            tasks[ti]()
                            ti += 1
                    if prev is not None:
                        emit_norm(prev[0], prev[1], prev[2], a2, rsl)
                    prev = (aps, b, mc, p2buf)
                # drain last unit of this rows-chunk
                paps, pb, pmc, pp2 = prev
                for si in range(NSC):
                    emit_pv(paps, pb, pmc, si, pp2[si])
                emit_norm(paps, pb, pmc, a2, rsl)
                while ti < len(tasks):
                    tasks[ti]()
                    ti += 1
                pending = (a2, r)
            # tail: attention pools released; a 6-bank pool and an even
            # DVE/Act rotation keep the final out-projection copy-unbound
            attctx.close()
            psT = cctx.enter_context(tc.tile_pool(name="psT", bufs=6,
                                                  space="PSUM"))
            for task in make_outproj_tasks(pending[0], pending[1], pool=psT,
                                           rot=[nc.vector, nc.scalar],
                                           tag="ops2"):
                task()
        pre1ctx.close()

    nc.compile()
    return nc


def _prep_inputs(inputs):
    """Host-side shard + transpose. Returns in_maps for 8 cores."""
    f32 = np.float32
    t = {k: np.asarray(v) for k, v in inputs.items()}
    x_full = {"t": t["trend_emb"].reshape(B * L, D).astype(f32),
              "d": t["detail_emb"].reshape(B * L, D).astype(f32)}
    pT_full = {"t": np.ascontiguousarray(t["trend_proto"].astype(f32).T).astype(bf16),
               "d": np.ascontiguousarray(t["detail_proto"].astype(f32).T).astype(bf16)}
    cat = np.concatenate([x_full["t"], x_full["d"]], axis=1)  # (B*L, 2D)
    w1_bf = t["g_W1"].astype(bf16)
    w2_bf = t["g_W2"].astype(bf16)
    gb1_f = np.ascontiguousarray(t["g_b1"].astype(f32).reshape(KD, 128).T)
    gb2_f = t["g_b2"].astype(f32).reshape(1, 1)
    W = {("q", "t"): t["t_Wq"], ("q", "d"): t["d_Wq"],
         ("k", "t"): t["t_Wk"], ("k", "d"): t["d_Wk"],
         ("v", "t"): t["t_Wv"], ("v", "d"): t["d_Wv"],
         ("o", "t"): t["t_Wo"], ("o", "d"): t["d_Wo"]}
    bias = {("q", "t"): t["t_bq"], ("q", "d"): t["d_bq"],
            ("k", "t"): t["t_bk"], ("k", "d"): t["d_bk"],
            ("v", "t"): t["t_bv"], ("v", "d"): t["d_bv"],
            ("o", "t"): t["t_bo"], ("o", "d"): t["d_bo"]}

    in_maps = []
    for core in range(8):
        rg, hg = divmod(core, HG)
        rows = slice(rg * R, (rg + 1) * R)
        hsl = slice(hg * HEC, (hg + 1) * HEC)
        gsl = slice(core * GR, (core + 1) * GR)
        m = {}
        for b in "td":
            m[f"xT_{b}"] = np.ascontiguousarray(x_full[b][rows].T).astype(bf16)
            m[f"pT_{b}"] = pT_full[b]
            m[f"wq_{b}"] = np.ascontiguousarray(W[("q", b)][:, hsl]).astype(bf16)
            m[f"wk_{b}"] = np.ascontiguousarray(W[("k", b)][:, hsl]).astype(bf16)
            m[f"wv_{b}"] = np.ascontiguousarray(W[("v", b)][:, hsl]).astype(bf16)
        m["wo"] = np.vstack([W[("o", "t")][hsl, :], W[("o", "d")][hsl, :]]).astype(bf16)
        m["w1"] = w1_bf
        m["w2"] = w2_bf
        m["gxT"] = np.ascontiguousarray(cat[gsl].T).astype(bf16)
        m["bq2"] = np.stack([bias[("q", "t")][hsl][0:128], bias[("q", "t")][hsl][128:256],
                             bias[("q", "d")][hsl][0:128], bias[("q", "d")][hsl][128:256]],
                            axis=1).astype(f32)
        m["bk2"] = np.stack([bias[("k", "t")][hsl][0:128], bias[("k", "t")][hsl][128:256],
                             bias[("k", "d")][hsl][0:128], bias[("k", "d")][hsl][128:256]],
                            axis=1).astype(f32)
        m["bv"] = np.concatenate([bias[("v", "t")][hsl],
                                  bias[("v", "d")][hsl]])[None, :].astype(bf16)
        m["gb1"] = gb1_f
        m["gb2"] = gb2_f
        in_maps.append(m)
    bo = (np.asarray(bias[("o", "t")], f32), np.asarray(bias[("o", "d")], f32))
    return in_maps, bo


def kernel(**inputs):
    global LAST_RESULTS
    import os
    from concourse.bass_utils import run_bass_kernel_spmd

    in_maps, (bo_t, bo_d) = _prep_inputs(inputs)
    if "nc" not in _CACHE:
        _CACHE["nc"] = _build()
    nc = _CACHE["nc"]

    trace = bool(os.environ.get("KERNEL_TRACE"))
    res = run_bass_kernel_spmd(
        nc, in_maps, list(range(8)),
        trace=trace, trace_cores=list(range(8)) if trace else None)
    LAST_RESULTS = res

    f32 = np.float32
    g = np.concatenate([np.asarray(res.results[c]["gate"]).reshape(GR)
                        for c in range(8)]).astype(f32)[:, None]   # (B*L, 1)
    out = np.empty((RG, R, DLLM), f32)
    for rg in range(RG):
        Pt = res.results[rg * HG]["out_t"].astype(f32)
        Pd = res.results[rg * HG]["out_d"].astype(f32)
        for hg in range(1, HG):
            Pt += res.results[rg * HG + hg]["out_t"].astype(f32)
            Pd += res.results[rg * HG + hg]["out_d"].astype(f32)
        grg = g[rg * R:(rg + 1) * R]
        out[rg] = grg * Pt + (1.0 - grg) * Pd
        if np.any(bo_t) or np.any(bo_d):
            out[rg] += grg * bo_t[None, :] + (1.0 - grg) * bo_d[None, :]
    return out.reshape(B, L, DLLM)
